# revision 18
# baseline (speedup 1.0000x reference)
"""Trainium2 Bass kernel for a 7-layer ternary-weight (BitNet) 1D conv
feature extractor with exact-erf GELU after each layer.

Contract: kernel(**inputs) takes the FULL inputs from setup_inputs()
(x: [8, 160000] f32, w0..w6 / b0..b6 conv params) and returns the full
output [8, 256, 500] f32.

Strategy: data-parallel over batch, one batch element per NeuronCore.
Weights are ternarized on host (sign in {-1,0,1}, exact in fp16; the
per-tensor absmean scale folds into the GELU's scale operand).

Single interleaved pipeline over all 7 layers, engineered to keep both
bottleneck engines (PE ~matmul cycles, ACT ~1 elem/cycle/lane GELU)
busy simultaneously:
 - L0 (contraction 10): 4x row-tiled matmuls (tile_position rows
   0/32/64/96) run concurrently, quartering L0's PE time. The host
   preps a phase buffer whose partition blocks hold the 4 time-slices.
 - Channels 128:192 of L1-L3 ("chunk1") are produced PARITY-SPLIT:
   even outputs t=2v via array cols 0:64, odd t=2v-1 via cols 64:128,
   col-tiled so the two accumulation chains run concurrently (half the
   PE cost of full-width 64-chan matmuls) and one full-lane GELU covers
   both. Buffer Bp[p<64, v] = out[128+p, 2v], Bp[64+p, v] =
   out[128+p, 2v-1].
 - Consumers read Bp with a "comb" matmul (contraction 128 = tap1 on
   even rows + tap0 on odd rows, contiguous columns) plus a 64-row
   "tap2" matmul at array rows 64:128 - no shifted-duplicate DMAs.
 - GELU calls are ~2048 wide from 4-bank PSUM tiles (2 rotating
   [128,2048] tiles = all 8 banks) to amortize ACT call overhead.
"""

import numpy as np

# (in_ch, out_ch, kernel, stride, pad) - fixed problem geometry
LAYERS = [(1, 128, 10, 5, 4), (128, 192, 3, 2, 1), (192, 192, 3, 2, 1),
          (192, 192, 3, 2, 1), (192, 256, 3, 2, 1), (256, 256, 4, 2, 1),
          (256, 256, 4, 2, 1)]
T_IN = 160000
LOUT = [32000, 16000, 8000, 4000, 2000, 1000, 500]
N_CORES = 8
NT = 512        # one fp32 PSUM bank
UW = 2048       # unit width (4 PSUM banks, one GELU call)


def _groups(i):
    """Accumulation groups of layer i>=1 (consumption form).
    ("full", k, ti): contraction 128 over channels 128*ti..+128 at tap k.
    ("comb",): contraction 128 on Bp: rows 0:64 = tap1, rows 64:128 = tap0.
    ("tap2",): contraction 64 at array rows 64:128 on Bp odd half."""
    cin, cout, k, s, p = LAYERS[i]
    if cin <= 128:
        return [("full", kk, 0) for kk in range(k)]
    if cin == 192:
        return ([("full", kk, 0) for kk in range(3)]
                + [("comb",), ("tap2",)])
    return [("full", kk, ti) for ti in (0, 1) for kk in range(k)]


def _layout():
    wcols = {0: 0}
    tot = 128
    for i in range(1, 7):
        wcols[i] = tot
        tot += len(_groups(i)) * LAYERS[i][1]
    bcols = {(0, 0): 0}
    nb = 2
    for i in range(1, 7):
        for mi in (0, 1):
            bcols[(i, mi)] = nb
            nb += 2
    return wcols, tot, bcols, nb


def _pack_host(ws, bs):
    """Ternarize weights; pack signs (fp16) and bias+scale (fp32)."""
    wcols, tot, bcols, nb = _layout()
    wpk = np.zeros((128, tot), np.float16)
    bpk = np.zeros((128, nb), np.float32)
    for i, (cin, cout, k, s, p) in enumerate(LAYERS):
        w = np.asarray(ws[i], np.float32)
        scale = max(float(np.mean(np.abs(w))), 1e-5)
        sign = np.clip(np.round(w / scale), -1.0, 1.0)  # [cout, cin, k]
        base = wcols[i]
        if i == 0:
            for q in range(4):
                wpk[32 * q:32 * q + 10, 0:128] = \
                    sign[:, 0, :].T.astype(np.float16)
        else:
            for gi, g in enumerate(_groups(i)):
                c = base + gi * cout
                if g[0] == "full":
                    kk, ti = g[1], g[2]
                    wpk[0:128, c:c + cout] = \
                        sign[:, 128 * ti:128 * ti + 128, kk].T
                elif g[0] == "comb":
                    wpk[0:64, c:c + cout] = sign[:, 128:192, 1].T
                    wpk[64:128, c:c + cout] = sign[:, 128:192, 0].T
                else:  # tap2
                    wpk[64:128, c:c + cout] = sign[:, 128:192, 2].T
        b = np.asarray(bs[i], np.float32)
        if i == 0:
            bpk[0:128, 0] = b
            bpk[0:128, 1] = scale
        else:
            for mi in (0, 1):
                c = bcols[(i, mi)]
                if cout == 192 and mi == 1:     # parity chunk: dup 64 rows
                    bpk[0:64, c] = b[128:192]
                    bpk[64:128, c] = b[128:192]
                else:
                    bpk[0:128, c] = b[128 * mi:128 * mi + 128]
                bpk[0:128, c + 1] = scale
    return wpk, bpk


def _prep_x(xb):
    """Per-core L0 input phases, blocked for 4x row tiling.
    xr4[10s+j, 1024c + 512b + w] = xpad[5*(4096c + 2048b + 512s + w) + j]."""
    xpad = np.zeros(T_IN + 16, np.float16)
    xpad[4:4 + T_IN] = xb.astype(np.float16)
    L = LOUT[0]
    xr = np.empty((10, L), np.float16)
    for j in range(10):
        xr[j, :] = xpad[j:j + 5 * L:5]
    xr4 = np.zeros((40, 8192), np.float16)
    for c in range(8):
        for b in range(2):
            for s in range(4):
                t0 = 4096 * c + 2048 * b + 512 * s
                if t0 >= L:
                    continue
                w = min(512, L - t0)
                xr4[10 * s:10 * s + 10, 1024 * c + 512 * b:
                    1024 * c + 512 * b + w] = xr[:, t0:t0 + w]
    return xr4


_CACHE = {}


def _build():
    """Build + compile the Bass program (weight-data-independent)."""
    if "nc" in _CACHE:
        return _CACHE["nc"]
    from concourse import bacc
    import concourse.mybir as mybir
    import concourse.tile as tile

    F16 = mybir.dt.float16
    F32 = mybir.dt.float32
    GELU = mybir.ActivationFunctionType.Gelu
    wcols, tot, bcols, nb = _layout()

    nc = bacc.Bacc("TRN2")
    xr_d = nc.dram_tensor("xr", [40, 8192], F16, kind="ExternalInput")
    wp_d = nc.dram_tensor("wp", [128, tot], F16, kind="ExternalInput")
    bp_d = nc.dram_tensor("bp", [128, nb], F32, kind="ExternalInput")
    y_d = nc.dram_tensor("y", [256, 500], F32, kind="ExternalOutput")

    import os as _os
    tap_shapes = {"a0": 32002, "A1": 16004, "Bp1": 8001, "A2": 8004,
                  "Bp2": 4001, "A3": 4004, "Bp3": 2001, "A4a": 2004,
                  "A4b": 2004, "A5a": 1004, "A5b": 1004}
    _CACHE["taps"] = {
        nm: nc.dram_tensor(f"tap_{nm}", [128, tap_shapes[nm]], F16,
                           kind="ExternalOutput")
        for nm in _os.environ.get("BITCONV_TAPS", "").split(",") if nm}

    with tile.TileContext(nc) as tc:
        pools = []

        def mkpool(name, bufs=1, space="SBUF"):
            p = tc.alloc_tile_pool(name=name, bufs=bufs, space=space)
            pools.append(p)
            return p

        wpool = mkpool("wpool")
        wt = wpool.tile([128, tot], F16, name="wt")
        bt = wpool.tile([128, nb], F32, name="bt")
        spool = mkpool("spool")
        scratch = spool.tile([128, 512], F16, name="scratch")
        stage = spool.tile([128, 1000], F32, name="stage")

        xpool = mkpool("xpool", bufs=8)

        # activations: chunk0 (interleaved, [128, L+4], data col = 1+u)
        # and parity chunk1 buffers Bp ([128, L/2+1])
        apool = mkpool("apool")
        a0 = apool.tile([128, LOUT[0] + 2], F16, name="a0")
        A = {0: [a0]}
        Bp = {}
        for i in range(1, 6):
            cout, lout = LAYERS[i][1], LOUT[i]
            if cout == 192:
                A[i] = [apool.tile([128, lout + 4], F16, name=f"A{i}")]
                Bp[i] = apool.tile([128, lout // 2 + 1], F16, name=f"Bp{i}")
            else:
                A[i] = [apool.tile([128, lout + 4], F16, name=f"A{i}_{m}")
                        for m in (0, 1)]

        # scratch memset first (gpsimd - earliest-booting engine) so junk
        # warm-up matmuls can start ASAP; pad memsets on vector after.
        nc.gpsimd.memset(scratch[:, :], 0.0)
        nc.vector.memset(a0[:, 0:1], 0.0)
        nc.vector.memset(a0[:, LOUT[0] + 1:LOUT[0] + 2], 0.0)
        for i in range(1, 6):
            lout = LOUT[i]
            for t in A[i]:
                nc.vector.memset(t[:, 0:1], 0.0)
                nc.vector.memset(t[:, lout + 1:lout + 3], 0.0)

        pspool = tc.alloc_tile_pool(name="pspool", bufs=2, space="PSUM")

        def junk_mms(n):
            jp = pspool.tile([128, UW], F32, name="ps", tag="ps")
            for _ in range(n):
                nc.tensor.matmul(jp[:, 0:NT], scratch[0:128, 0:128],
                                 scratch[:, 0:NT], start=True, stop=True)

        # ---------------- unit emitters ----------------
        # emission-order guards: a unit must only be emitted after every
        # producer of the columns it reads (the Tile framework can only
        # synchronize against already-emitted instructions; violating this
        # reads stale SBUF and is masked on warm re-runs).
        prodA = {(i, mi): 0 for i in range(6) for mi in (0, 1)}
        prodBp = {i: 0 for i in (1, 2, 3)}

        def need_A(i, ti, ucount):
            have = prodA[(i, ti)]
            assert have >= min(ucount, LOUT[i]), \
                f"A{i}[{ti}]: need {ucount} cols, produced {have}"

        def need_Bp(i, vcount):
            have = prodBp[i]
            assert have >= min(vcount, LOUT[i] // 2 + 1), \
                f"Bp{i}: need {vcount} cols, produced {have}"

        xts = {}

        def emit_xchunk(c):
            xt = xpool.tile([106, 1024], F16, tag="xt", name=f"xt{c}")
            for s in range(4):
                nc.sync.dma_start(
                    out=xt[32 * s:32 * s + 10, :],
                    in_=xr_d.ap()[10 * s:10 * s + 10,
                                  1024 * c:1024 * c + 1024])
            xts[c] = xt

        def emit_u0(iu):
            """L0 unit: outputs [2048*iu, +2048), 4x row-tiled."""
            t0 = 2048 * iu
            n = min(UW, LOUT[0] - t0)
            xt, b = xts[iu // 2], iu % 2
            ps = pspool.tile([128, UW], F32, name="ps", tag="ps")
            for s in range(4):
                o = 512 * s
                if o >= n:
                    break
                w = min(NT, n - o)
                nc.tensor.matmul(
                    ps[:, o:o + w], wt[32 * s:32 * s + 10, 0:128],
                    xt[32 * s:32 * s + 10, 512 * b:512 * b + w],
                    start=True, stop=True, tile_position=(32 * s, 0))
            nc.scalar.activation(a0[0:128, 1 + t0:1 + t0 + n],
                                 ps[0:128, 0:n], GELU,
                                 bias=bt[:, 0:1], scale=bt[:, 1:2])
            prodA[(0, 0)] = t0 + n

        def strided(src, p0, p1, c, step, n):
            """src[p0:p1, c : c+step*n : step] with exact slice end."""
            return src[p0:p1, c:c + step * (n - 1) + 1:step]

        def rhs_cons(i, g, t, w):
            """Consumption rhs (stride 2) for layer i group g at outputs
            [t, t+w)."""
            if g[0] == "full":
                kk, ti = g[1], g[2]
                src = A[i - 1][ti]
                c = 2 * t + kk
                return strided(src, 0, 128, c, 2, w)
            src = Bp[i - 1]
            if g[0] == "comb":
                return src[0:128, t:t + w]
            return src[64:128, t + 1:t + 1 + w]

        def emit_fw(i, mi, t0, n):
            """Full-width unit: layer i out-channels 128*mi..+128 over
            outputs [t0, t0+n)."""
            cout = LAYERS[i][1]
            gs = _groups(i)
            na = len(gs)
            base = wcols[i]
            kmax = LAYERS[i][2] - 1
            for ti in set(g[2] for g in gs if g[0] == "full"):
                need_A(i - 1, ti, 2 * (t0 + n - 1) + kmax)
            if any(g[0] == "comb" for g in gs):
                need_Bp(i - 1, t0 + n + 1)
            ps = pspool.tile([128, UW], F32, name="ps", tag="ps")
            # weight-outer: consecutive matmuls share the stationary operand
            for a, g in enumerate(gs):
                wc = base + a * cout + 128 * mi
                if g[0] == "tap2":
                    lhsT = wt[64:128, wc:wc + 128]
                else:
                    lhsT = wt[0:128, wc:wc + 128]
                for o in range(0, n, NT):
                    w = min(NT, n - o)
                    nc.tensor.matmul(ps[0:128, o:o + w], lhsT,
                                     rhs_cons(i, g, t0 + o, w),
                                     start=(a == 0), stop=(a == na - 1))
            bc = bcols[(i, mi)]
            if i == 6:
                dst = stage[0:128, 500 * mi:500 * mi + n]
            else:
                dst = A[i][mi][0:128, 1 + t0:1 + t0 + n]
            nc.scalar.activation(dst, ps[0:128, 0:n], GELU,
                                 bias=bt[0:128, bc:bc + 1],
                                 scale=bt[0:128, bc + 1:bc + 2])
            if i < 6:
                prodA[(i, mi)] = t0 + n

        def emit_par(i, v0, v1):
            """Parity unit: layer i channels 128:192, Bp_i cols [v0, v1).
            Even chain (psum rows 0:64): t=2v; odd chain (rows 64:128):
            t=2v-1; col-tiled concurrent accumulation chains."""
            L = LOUT[i]
            gs = _groups(i)
            na = len(gs)
            base = wcols[i]
            cout = LAYERS[i][1]
            n = v1 - v0
            need_A(i - 1, 0, 4 * (v1 - 1) + 2)
            if any(g[0] == "comb" for g in gs):
                need_Bp(i - 1, 2 * v1)
            ps = pspool.tile([128, UW], F32, name="ps", tag="ps")
            if v0 == 0:
                nc.vector.memset(ps[64:128, 0:1], 0.0)  # t=-1 slot
            if v1 == L // 2 + 1:
                nc.vector.memset(ps[0:64, n - 1:n], 0.0)  # t=L slot
            # weight-outer; even/odd chains share lhsT and run col-tiled
            for a, g in enumerate(gs):
                wc = base + a * cout + 128
                if g[0] == "tap2":
                    lhsT = wt[64:128, wc:wc + 64]
                else:
                    lhsT = wt[0:128, wc:wc + 64]
                st, sp = (a == 0), (a == na - 1)
                for o in range(0, n, NT):
                    w = min(NT, n - o)
                    v = v0 + o
                    we = max(0, min(v + w, L // 2) - v)  # even valid width
                    so = 1 if v == 0 else 0              # odd start shift
                    vo = v + so
                    wo = w - so
                    if we > 0:
                        if g[0] == "full":
                            rhs = strided(A[i - 1][0], 0, 128,
                                          4 * v + g[1], 4, we)
                        elif g[0] == "comb":
                            rhs = strided(Bp[i - 1], 0, 128, 2 * v, 2, we)
                        else:
                            rhs = strided(Bp[i - 1], 64, 128,
                                          2 * v + 1, 2, we)
                        nc.tensor.matmul(ps[0:64, o:o + we], lhsT, rhs,
                                         start=st, stop=sp)
                    if wo > 0:
                        if g[0] == "full":
                            rhs = strided(A[i - 1][0], 0, 128,
                                          4 * vo + g[1] - 2, 4, wo)
                        elif g[0] == "comb":
                            rhs = strided(Bp[i - 1], 0, 128,
                                          2 * vo - 1, 2, wo)
                        else:
                            rhs = strided(Bp[i - 1], 64, 128, 2 * vo, 2, wo)
                        nc.tensor.matmul(ps[64:128, o + so:o + w], lhsT, rhs,
                                         start=st, stop=sp)
            bc = bcols[(i, 1)]
            nc.scalar.activation(Bp[i][0:128, v0:v1], ps[0:128, 0:n], GELU,
                                 bias=bt[0:128, bc:bc + 1],
                                 scale=bt[0:128, bc + 1:bc + 2])
            if v0 == 0:
                nc.vector.memset(Bp[i][64:128, 0:1], 0.0)  # zero pad t=-1
            prodBp[i] = v1

        # ---------------- schedule (the weave) ----------------
        # all x chunks prefetched upfront: DMA completion latency is ~7.5us
        # and the bulk-weight DMA competes on the same engines.
        for c in range(8):
            emit_xchunk(c)
        nc.sync.dma_start(out=wt[0:106, 0:128], in_=wp_d.ap()[0:106, 0:128])
        nc.sync.dma_start(out=bt[:, :], in_=bp_d.ap())
        # bulk weights via SWDGE (gpsimd). Piece 1 (L1) posts immediately;
        # pieces 2/3 are gated behind a0-progress reads so their descriptors
        # don't contend with early traffic (gpsimd queue is FIFO).
        l1end = wcols[2]
        l3end = wcols[4]
        nc.gpsimd.dma_start(out=wt[:, 128:l1end],
                            in_=wp_d.ap()[:, 128:l1end])
        # PE warm-up: cover the ~16.5us until the first x data is
        # consumable, so HAM reaches 8/8 and real matmuls run at 2.4 GHz.
        junk_mms(12)
        emit_u0(0)
        junk_mms(9)
        emit_u0(1)
        junk_mms(9)
        emit_fw(1, 0, 0, UW)            # U1m0[0]
        emit_u0(2)
        emit_u0(3)
        emit_fw(1, 0, UW, UW)           # U1m0[1]
        emit_par(1, 0, 2048)            # U1m1[0]
        nc.gpsimd.tensor_copy(scratch[0:1, 0:1], a0[0:1, 4000:4001])
        nc.gpsimd.dma_start(out=wt[:, l1end:l3end],
                            in_=wp_d.ap()[:, l1end:l3end])
        emit_u0(4)
        emit_u0(5)
        emit_fw(1, 0, 2 * UW, UW)       # U1m0[2]
        emit_u0(6)
        emit_u0(7)
        emit_fw(1, 0, 3 * UW, UW)       # U1m0[3]
        emit_par(1, 2048, 4096)         # U1m1[1]
        nc.gpsimd.tensor_copy(scratch[0:1, 1:2], a0[0:1, 12000:12001])
        nc.gpsimd.dma_start(out=wt[:, l3end:tot],
                            in_=wp_d.ap()[:, l3end:tot])
        emit_u0(8)
        emit_u0(9)
        emit_fw(1, 0, 4 * UW, UW)       # U1m0[4]
        emit_fw(2, 0, 0, UW)            # U2m0[0]
        emit_u0(10)
        emit_u0(11)
        emit_fw(1, 0, 5 * UW, UW)       # U1m0[5]
        emit_par(1, 4096, 6144)         # U1m1[2]
        emit_u0(12)
        emit_u0(13)
        emit_fw(1, 0, 6 * UW, UW)       # U1m0[6]
        emit_fw(2, 0, UW, UW)           # U2m0[1]
        emit_u0(14)
        emit_u0(15)
        emit_fw(1, 0, 7 * UW, 16000 - 7 * UW)   # U1m0[7]
        emit_par(1, 6144, 8001)         # U1m1[3]
        emit_fw(2, 0, 2 * UW, UW)       # U2m0[2]
        emit_par(2, 0, 2048)            # U2m1[0]
        emit_fw(2, 0, 3 * UW, 8000 - 3 * UW)    # U2m0[3]
        emit_par(2, 2048, 4001)         # U2m1[1]
        emit_fw(3, 0, 0, UW)            # U3m0[0]
        emit_fw(3, 0, UW, 4000 - UW)    # U3m0[1]
        emit_par(3, 0, 2001)            # U3m1[0]
        emit_fw(4, 0, 0, 2000)
        emit_fw(4, 1, 0, 2000)
        emit_fw(5, 0, 0, 1000)
        emit_fw(5, 1, 0, 1000)
        emit_fw(6, 0, 0, 500)
        nc.sync.dma_start(out=y_d.ap()[0:128, :], in_=stage[:, 0:500])
        emit_fw(6, 1, 0, 500)
        nc.sync.dma_start(out=y_d.ap()[128:256, :], in_=stage[:, 500:1000])
        for nm, t in _CACHE.get("taps", {}).items():
            buf = {"a0": a0, "A1": A[1][0], "Bp1": Bp[1], "A2": A[2][0],
                   "Bp2": Bp[2], "A3": A[3][0], "Bp3": Bp[3],
                   "A4a": A[4][0], "A4b": A[4][1],
                   "A5a": A[5][0], "A5b": A[5][1]}[nm]
            nc.sync.dma_start(out=t.ap(), in_=buf[:, :])
        pspool.release()
        for p in reversed(pools):
            p.release()

    nc.compile()
    _CACHE["nc"] = nc
    return nc


def kernel(x, w0, b0, w1, b1, w2, b2, w3, b3, w4, b4, w5, b5, w6, b6):
    import os
    from concourse.bass_utils import run_bass_kernel_spmd

    ws = [w0, w1, w2, w3, w4, w5, w6]
    bs = [b0, b1, b2, b3, b4, b5, b6]
    wpk, bpk = _pack_host(ws, bs)
    x = np.asarray(x, np.float32)
    in_maps = [{"xr": _prep_x(x[b]), "wp": wpk, "bp": bpk}
               for b in range(N_CORES)]
    nc = _build()
    trace = bool(os.environ.get("BITCONV_TRACE"))
    res = run_bass_kernel_spmd(nc, in_maps, core_ids=list(range(N_CORES)),
                               trace=trace)
    if trace:
        print(f"HW exec time: {res.exec_time_ns} ns")
        _CACHE["last_results"] = res
    return np.stack([res.results[b]["y"] for b in range(N_CORES)], axis=0)


# revision 26
# speedup vs baseline: 1.1896x; 1.1896x over previous
"""Trainium2 Bass kernel for a 7-layer ternary-weight (BitNet) 1D conv
feature extractor with exact-erf GELU after each layer.

Contract: kernel(**inputs) takes the FULL inputs from setup_inputs()
(x: [8, 160000] f32, w0..w6 / b0..b6 conv params) and returns the full
output [8, 256, 500] f32.

Strategy: data-parallel over batch, one batch element per NeuronCore.
Weights are ternarized on host (sign in {-1,0,1}, exact in fp16; the
per-tensor absmean scale folds into the GELU's scale operand).

Single interleaved pipeline over all 7 layers, engineered to keep both
bottleneck engines (PE ~matmul cycles, ACT ~1 elem/cycle/lane GELU)
busy simultaneously:
 - L0 (contraction 10): 4x row-tiled matmuls (tile_position rows
   0/32/64/96) run concurrently, quartering L0's PE time. The host
   preps a phase buffer whose partition blocks hold the 4 time-slices.
 - Channels 128:192 of L1-L3 ("chunk1") are produced PARITY-SPLIT:
   even outputs t=2v via array cols 0:64, odd t=2v-1 via cols 64:128,
   col-tiled so the two accumulation chains run concurrently (half the
   PE cost of full-width 64-chan matmuls) and one full-lane GELU covers
   both. Buffer Bp[p<64, v] = out[128+p, 2v], Bp[64+p, v] =
   out[128+p, 2v-1].
 - Consumers read Bp with a "comb" matmul (contraction 128 = tap1 on
   even rows + tap0 on odd rows, contiguous columns) plus a 64-row
   "tap2" matmul at array rows 64:128 - no shifted-duplicate DMAs.
 - GELU calls are ~2048 wide from 4-bank PSUM tiles (2 rotating
   [128,2048] tiles = all 8 banks) to amortize ACT call overhead.
"""

import numpy as np

# (in_ch, out_ch, kernel, stride, pad) - fixed problem geometry
LAYERS = [(1, 128, 10, 5, 4), (128, 192, 3, 2, 1), (192, 192, 3, 2, 1),
          (192, 192, 3, 2, 1), (192, 256, 3, 2, 1), (256, 256, 4, 2, 1),
          (256, 256, 4, 2, 1)]
T_IN = 160000
LOUT = [32000, 16000, 8000, 4000, 2000, 1000, 500]
N_CORES = 8
NT = 512        # one fp32 PSUM bank
UW = 2048       # unit width (4 PSUM banks, one GELU call)


def _groups(i):
    """Accumulation groups of layer i>=1 (consumption form).
    ("full", k, ti): contraction 128 over channels 128*ti..+128 at tap k.
    ("comb",): contraction 128 on Bp: rows 0:64 = tap1, rows 64:128 = tap0.
    ("tap2",): contraction 64 at array rows 64:128 on Bp odd half."""
    cin, cout, k, s, p = LAYERS[i]
    if cin <= 128:
        return [("full", kk, 0) for kk in range(k)]
    if cin == 192:
        return ([("full", kk, 0) for kk in range(3)]
                + [("comb",), ("tap2",)])
    return [("full", kk, ti) for ti in (0, 1) for kk in range(k)]


def _layout():
    wcols = {0: 0}
    tot = 128
    for i in range(1, 7):
        wcols[i] = tot
        tot += len(_groups(i)) * LAYERS[i][1]
    bcols = {(0, 0): 0}
    nb = 2
    for i in range(1, 7):
        for mi in (0, 1):
            bcols[(i, mi)] = nb
            nb += 2
    return wcols, tot, bcols, nb


def _pack_host(ws, bs):
    """Ternarize weights; pack signs (fp16) and bias+scale (fp32)."""
    wcols, tot, bcols, nb = _layout()
    wpk = np.zeros((128, tot), np.float16)
    bpk = np.zeros((128, nb), np.float32)
    for i, (cin, cout, k, s, p) in enumerate(LAYERS):
        w = np.asarray(ws[i], np.float32)
        scale = max(float(np.mean(np.abs(w))), 1e-5)
        sign = np.clip(np.round(w / scale), -1.0, 1.0)  # [cout, cin, k]
        base = wcols[i]
        if i == 0:
            for q in range(4):
                wpk[32 * q:32 * q + 10, 0:128] = \
                    sign[:, 0, :].T.astype(np.float16)
        else:
            for gi, g in enumerate(_groups(i)):
                c = base + gi * cout
                if g[0] == "full":
                    kk, ti = g[1], g[2]
                    wpk[0:128, c:c + cout] = \
                        sign[:, 128 * ti:128 * ti + 128, kk].T
                elif g[0] == "comb":
                    wpk[0:64, c:c + cout] = sign[:, 128:192, 1].T
                    wpk[64:128, c:c + cout] = sign[:, 128:192, 0].T
                else:  # tap2
                    wpk[64:128, c:c + cout] = sign[:, 128:192, 2].T
        b = np.asarray(bs[i], np.float32)
        if i == 0:
            bpk[0:128, 0] = b
            bpk[0:128, 1] = scale
        else:
            for mi in (0, 1):
                c = bcols[(i, mi)]
                if cout == 192 and mi == 1:     # parity chunk: dup 64 rows
                    bpk[0:64, c] = b[128:192]
                    bpk[64:128, c] = b[128:192]
                else:
                    bpk[0:128, c] = b[128 * mi:128 * mi + 128]
                bpk[0:128, c + 1] = scale
    return wpk, bpk


def _prep_x(xb):
    """Per-core L0 input phases, blocked for 4x row tiling.
    xr4[10s+j, 1024c + 512b + w] = xpad[5*(4096c + 2048b + 512s + w) + j]."""
    xpad = np.zeros(T_IN + 16, np.float16)
    xpad[4:4 + T_IN] = xb.astype(np.float16)
    L = LOUT[0]
    xr = np.empty((10, L), np.float16)
    for j in range(10):
        xr[j, :] = xpad[j:j + 5 * L:5]
    xr4 = np.zeros((40, 8192), np.float16)
    for c in range(8):
        for b in range(2):
            for s in range(4):
                t0 = 4096 * c + 2048 * b + 512 * s
                if t0 >= L:
                    continue
                w = min(512, L - t0)
                xr4[10 * s:10 * s + 10, 1024 * c + 512 * b:
                    1024 * c + 512 * b + w] = xr[:, t0:t0 + w]
    return xr4


_CACHE = {}


def _build():
    """Build + compile the Bass program (weight-data-independent)."""
    if "nc" in _CACHE:
        return _CACHE["nc"]
    from concourse import bacc
    import concourse.mybir as mybir
    import concourse.tile as tile

    F16 = mybir.dt.float16
    F32 = mybir.dt.float32
    GELU = mybir.ActivationFunctionType.Gelu
    wcols, tot, bcols, nb = _layout()

    nc = bacc.Bacc("TRN2")
    xr_d = nc.dram_tensor("xr", [40, 8192], F16, kind="ExternalInput")
    wp_d = nc.dram_tensor("wp", [128, tot], F16, kind="ExternalInput")
    bp_d = nc.dram_tensor("bp", [128, nb], F32, kind="ExternalInput")
    y_d = nc.dram_tensor("y", [256, 500], F32, kind="ExternalOutput")

    import os as _os
    tap_shapes = {"a0": 32002, "A1": 16004, "Bp1": 8001, "A2": 8004,
                  "Bp2": 4001, "A3": 4004, "Bp3": 2001, "A4a": 2004,
                  "A4b": 2004, "A5a": 1004, "A5b": 1004}
    _CACHE["taps"] = {
        nm: nc.dram_tensor(f"tap_{nm}", [128, tap_shapes[nm]], F16,
                           kind="ExternalOutput")
        for nm in _os.environ.get("BITCONV_TAPS", "").split(",") if nm}

    with tile.TileContext(nc) as tc:
        pools = []

        def mkpool(name, bufs=1, space="SBUF"):
            p = tc.alloc_tile_pool(name=name, bufs=bufs, space=space)
            pools.append(p)
            return p

        wpool = mkpool("wpool")
        wt = wpool.tile([128, tot], F16, name="wt")
        bt = wpool.tile([128, nb], F32, name="bt")
        spool = mkpool("spool")
        scratch = spool.tile([128, 512], F16, name="scratch")
        stage = spool.tile([128, 1000], F32, name="stage")

        xpool = mkpool("xpool", bufs=3)

        # activations: chunk0 (interleaved, [128, L+4], data col = 1+u)
        # and parity chunk1 buffers Bp ([128, L/2+1])
        apool = mkpool("apool")
        a0 = apool.tile([128, LOUT[0] + 2], F16, name="a0")
        A = {0: [a0]}
        Bp = {}
        for i in range(1, 6):
            cout, lout = LAYERS[i][1], LOUT[i]
            if cout == 192:
                A[i] = [apool.tile([128, lout + 4], F16, name=f"A{i}")]
                Bp[i] = apool.tile([128, lout // 2 + 1], F16, name=f"Bp{i}")
            else:
                A[i] = [apool.tile([128, lout + 4], F16, name=f"A{i}_{m}")
                        for m in (0, 1)]

        # scratch memset first (gpsimd - earliest-booting engine) so junk
        # warm-up matmuls can start ASAP; pad memsets on vector after.
        nc.gpsimd.memset(scratch[:, :], 0.0)
        nc.vector.memset(a0[:, 0:1], 0.0)
        nc.vector.memset(a0[:, LOUT[0] + 1:LOUT[0] + 2], 0.0)
        for i in range(1, 6):
            lout = LOUT[i]
            for t in A[i]:
                nc.vector.memset(t[:, 0:1], 0.0)
                nc.vector.memset(t[:, lout + 1:lout + 3], 0.0)

        # 4 rotating 2-bank PSUM tiles: deep enough that the PE can run
        # ahead of the ACT queue (idle gaps stay under the ~3.4us HAM
        # re-throttle window, keeping the PE clock at 2.4 GHz).
        SUB = 1024
        pspool = tc.alloc_tile_pool(name="pspool", bufs=4, space="PSUM")

        def junk_mms(n):
            jp = pspool.tile([128, SUB], F32, name="ps", tag="ps")
            for _ in range(n):
                nc.tensor.matmul(jp[:, 0:NT], scratch[0:128, 0:128],
                                 scratch[:, 0:NT], start=True, stop=True)

        # ---------------- unit emitters ----------------
        # emission-order guards: a unit must only be emitted after every
        # producer of the columns it reads (the Tile framework can only
        # synchronize against already-emitted instructions; violating this
        # reads stale SBUF and is masked on warm re-runs).
        prodA = {(i, mi): 0 for i in range(6) for mi in (0, 1)}
        prodBp = {i: 0 for i in (1, 2, 3)}

        def need_A(i, ti, ucount):
            have = prodA[(i, ti)]
            assert have >= min(ucount, LOUT[i]), \
                f"A{i}[{ti}]: need {ucount} cols, produced {have}"

        def need_Bp(i, vcount):
            have = prodBp[i]
            assert have >= min(vcount, LOUT[i] // 2 + 1), \
                f"Bp{i}: need {vcount} cols, produced {have}"

        xts = {}

        def emit_xchunk(c):
            xt = xpool.tile([106, 1024], F16, tag="xt", name=f"xt{c}")
            for s in range(4):
                nc.sync.dma_start(
                    out=xt[32 * s:32 * s + 10, :],
                    in_=xr_d.ap()[10 * s:10 * s + 10,
                                  1024 * c:1024 * c + 1024])
            xts[c] = xt

        def emit_u0(iu):
            """L0 unit: outputs [2048*iu, +2048), 4x row-tiled, two
            1024-wide psum sub-units."""
            t0 = 2048 * iu
            n = min(UW, LOUT[0] - t0)
            xt, b = xts[iu // 2], iu % 2
            for half in range(2):
                hn = min(SUB, n - SUB * half)
                if hn <= 0:
                    break
                ps = pspool.tile([128, SUB], F32, name="ps", tag="ps")
                for s2 in range(2):
                    o = 512 * s2
                    if o >= hn:
                        break
                    s = 2 * half + s2
                    w = min(NT, hn - o)
                    nc.tensor.matmul(
                        ps[:, o:o + w], wt[32 * s:32 * s + 10, 0:128],
                        xt[32 * s:32 * s + 10, 512 * b:512 * b + w],
                        start=True, stop=True, tile_position=(32 * s, 0))
                nc.scalar.activation(
                    a0[0:128, 1 + t0 + SUB * half:1 + t0 + SUB * half + hn],
                    ps[0:128, 0:hn], GELU,
                    bias=bt[:, 0:1], scale=bt[:, 1:2])
            prodA[(0, 0)] = t0 + n

        def strided(src, p0, p1, c, step, n):
            """src[p0:p1, c : c+step*n : step] with exact slice end."""
            return src[p0:p1, c:c + step * (n - 1) + 1:step]

        def rhs_cons(i, g, t, w):
            """Consumption rhs (stride 2) for layer i group g at outputs
            [t, t+w)."""
            if g[0] == "full":
                kk, ti = g[1], g[2]
                src = A[i - 1][ti]
                c = 2 * t + kk
                return strided(src, 0, 128, c, 2, w)
            src = Bp[i - 1]
            if g[0] == "comb":
                return src[0:128, t:t + w]
            return src[64:128, t + 1:t + 1 + w]

        def emit_fw(i, mi, t0, n):
            """Full-width unit: layer i out-channels 128*mi..+128 over
            outputs [t0, t0+n)."""
            cout = LAYERS[i][1]
            gs = _groups(i)
            na = len(gs)
            base = wcols[i]
            kmax = LAYERS[i][2] - 1
            for ti in set(g[2] for g in gs if g[0] == "full"):
                need_A(i - 1, ti, 2 * (t0 + n - 1) + kmax)
            if any(g[0] == "comb" for g in gs):
                need_Bp(i - 1, t0 + n + 1)
            bc = bcols[(i, mi)]
            for sub in range(0, n, SUB):
                m = min(SUB, n - sub)
                ts = t0 + sub
                ps = pspool.tile([128, SUB], F32, name="ps", tag="ps")
                # weight-outer: consecutive matmuls share the stationary op
                for a, g in enumerate(gs):
                    wc = base + a * cout + 128 * mi
                    if g[0] == "tap2":
                        lhsT = wt[64:128, wc:wc + 128]
                    else:
                        lhsT = wt[0:128, wc:wc + 128]
                    for o in range(0, m, NT):
                        w = min(NT, m - o)
                        nc.tensor.matmul(ps[0:128, o:o + w], lhsT,
                                         rhs_cons(i, g, ts + o, w),
                                         start=(a == 0), stop=(a == na - 1))
                if i == 6:
                    dst = stage[0:128, 500 * mi:500 * mi + m]
                else:
                    dst = A[i][mi][0:128, 1 + ts:1 + ts + m]
                nc.scalar.activation(dst, ps[0:128, 0:m], GELU,
                                     bias=bt[0:128, bc:bc + 1],
                                     scale=bt[0:128, bc + 1:bc + 2])
            if i < 6:
                prodA[(i, mi)] = t0 + n

        def emit_par(i, v0, v1):
            """Parity unit: layer i channels 128:192, Bp_i cols [v0, v1).
            Even chain (psum rows 0:64): t=2v; odd chain (rows 64:128):
            t=2v-1; col-tiled concurrent accumulation chains."""
            L = LOUT[i]
            gs = _groups(i)
            na = len(gs)
            base = wcols[i]
            cout = LAYERS[i][1]
            n = v1 - v0
            need_A(i - 1, 0, 4 * (v1 - 1) + 2)
            if any(g[0] == "comb" for g in gs):
                need_Bp(i - 1, 2 * v1)
            bc = bcols[(i, 1)]
            for sub in range(0, n, SUB):
                m = min(SUB, n - sub)
                vs = v0 + sub
                ps = pspool.tile([128, SUB], F32, name="ps", tag="ps")
                if vs == 0:
                    nc.vector.memset(ps[64:128, 0:1], 0.0)  # t=-1 slot
                if vs + m == L // 2 + 1:
                    nc.vector.memset(ps[0:64, m - 1:m], 0.0)  # t=L slot
                # weight-outer; even/odd chains share lhsT, run col-tiled
                for a, g in enumerate(gs):
                    wc = base + a * cout + 128
                    if g[0] == "tap2":
                        lhsT = wt[64:128, wc:wc + 64]
                    else:
                        lhsT = wt[0:128, wc:wc + 64]
                    st, sp = (a == 0), (a == na - 1)
                    for o in range(0, m, NT):
                        w = min(NT, m - o)
                        v = vs + o
                        we = max(0, min(v + w, L // 2) - v)  # even width
                        so = 1 if v == 0 else 0              # odd shift
                        vo = v + so
                        wo = w - so
                        if we > 0:
                            if g[0] == "full":
                                rhs = strided(A[i - 1][0], 0, 128,
                                              4 * v + g[1], 4, we)
                            elif g[0] == "comb":
                                rhs = strided(Bp[i - 1], 0, 128,
                                              2 * v, 2, we)
                            else:
                                rhs = strided(Bp[i - 1], 64, 128,
                                              2 * v + 1, 2, we)
                            nc.tensor.matmul(ps[0:64, o:o + we], lhsT, rhs,
                                             start=st, stop=sp)
                        if wo > 0:
                            if g[0] == "full":
                                rhs = strided(A[i - 1][0], 0, 128,
                                              4 * vo + g[1] - 2, 4, wo)
                            elif g[0] == "comb":
                                rhs = strided(Bp[i - 1], 0, 128,
                                              2 * vo - 1, 2, wo)
                            else:
                                rhs = strided(Bp[i - 1], 64, 128,
                                              2 * vo, 2, wo)
                            nc.tensor.matmul(ps[64:128, o + so:o + w],
                                             lhsT, rhs, start=st, stop=sp)
                nc.scalar.activation(Bp[i][0:128, vs:vs + m],
                                     ps[0:128, 0:m], GELU,
                                     bias=bt[0:128, bc:bc + 1],
                                     scale=bt[0:128, bc + 1:bc + 2])
                if vs == 0:
                    nc.vector.memset(Bp[i][64:128, 0:1], 0.0)  # pad t=-1
            prodBp[i] = v1

        # ---------------- schedule (the weave) ----------------
        emit_xchunk(0)
        nc.sync.dma_start(out=wt[0:106, 0:128], in_=wp_d.ap()[0:106, 0:128])
        nc.sync.dma_start(out=bt[:, :], in_=bp_d.ap())
        emit_xchunk(1)
        # bulk weights via SWDGE (gpsimd). Piece 1 (L1) posts immediately;
        # pieces 2/3 are gated behind a0-progress reads so their descriptors
        # don't contend with early traffic (gpsimd queue is FIFO).
        l1end = wcols[2]
        l3end = wcols[4]
        nc.gpsimd.dma_start(out=wt[:, 128:l1end],
                            in_=wp_d.ap()[:, 128:l1end])
        # PE warm-up: cover the ~16.5us until the first x data is
        # consumable, so HAM reaches 8/8 and real matmuls run at 2.4 GHz.
        junk_mms(12)
        emit_u0(0)
        junk_mms(12)
        emit_u0(1)
        junk_mms(12)
        emit_fw(1, 0, 0, UW)            # U1m0[0]
        emit_xchunk(2)
        emit_u0(2)
        emit_u0(3)
        emit_fw(1, 0, UW, UW)           # U1m0[1]
        emit_par(1, 0, 2048)            # U1m1[0]
        nc.gpsimd.tensor_copy(scratch[0:1, 0:1], a0[0:1, 4000:4001])
        nc.gpsimd.dma_start(out=wt[:, l1end:l3end],
                            in_=wp_d.ap()[:, l1end:l3end])
        emit_xchunk(3)
        emit_u0(4)
        emit_u0(5)
        emit_fw(1, 0, 2 * UW, UW)       # U1m0[2]
        emit_xchunk(4)
        emit_u0(6)
        emit_u0(7)
        emit_fw(1, 0, 3 * UW, UW)       # U1m0[3]
        emit_par(1, 2048, 4096)         # U1m1[1]
        nc.gpsimd.tensor_copy(scratch[0:1, 1:2], a0[0:1, 12000:12001])
        nc.gpsimd.dma_start(out=wt[:, l3end:tot],
                            in_=wp_d.ap()[:, l3end:tot])
        emit_xchunk(5)
        emit_u0(8)
        emit_u0(9)
        emit_fw(1, 0, 4 * UW, UW)       # U1m0[4]
        emit_fw(2, 0, 0, UW)            # U2m0[0]
        emit_xchunk(6)
        emit_u0(10)
        emit_u0(11)
        emit_fw(1, 0, 5 * UW, UW)       # U1m0[5]
        emit_par(1, 4096, 6144)         # U1m1[2]
        emit_xchunk(7)
        emit_u0(12)
        emit_u0(13)
        emit_fw(1, 0, 6 * UW, UW)       # U1m0[6]
        emit_fw(2, 0, UW, UW)           # U2m0[1]
        emit_u0(14)
        emit_u0(15)
        emit_fw(1, 0, 7 * UW, 16000 - 7 * UW)   # U1m0[7]
        emit_par(1, 6144, 8001)         # U1m1[3]
        emit_fw(2, 0, 2 * UW, UW)       # U2m0[2]
        emit_par(2, 0, 2048)            # U2m1[0]
        emit_fw(2, 0, 3 * UW, 8000 - 3 * UW)    # U2m0[3]
        emit_par(2, 2048, 4001)         # U2m1[1]
        emit_fw(3, 0, 0, UW)            # U3m0[0]
        emit_fw(3, 0, UW, 4000 - UW)    # U3m0[1]
        emit_par(3, 0, 2001)            # U3m1[0]
        emit_fw(4, 0, 0, 2000)
        emit_fw(4, 1, 0, 2000)
        emit_fw(5, 0, 0, 1000)
        emit_fw(5, 1, 0, 1000)
        emit_fw(6, 0, 0, 500)
        nc.sync.dma_start(out=y_d.ap()[0:128, :], in_=stage[:, 0:500])
        emit_fw(6, 1, 0, 500)
        nc.sync.dma_start(out=y_d.ap()[128:256, :], in_=stage[:, 500:1000])
        for nm, t in _CACHE.get("taps", {}).items():
            buf = {"a0": a0, "A1": A[1][0], "Bp1": Bp[1], "A2": A[2][0],
                   "Bp2": Bp[2], "A3": A[3][0], "Bp3": Bp[3],
                   "A4a": A[4][0], "A4b": A[4][1],
                   "A5a": A[5][0], "A5b": A[5][1]}[nm]
            nc.sync.dma_start(out=t.ap(), in_=buf[:, :])
        pspool.release()
        for p in reversed(pools):
            p.release()

    nc.compile()
    _CACHE["nc"] = nc
    return nc


def kernel(x, w0, b0, w1, b1, w2, b2, w3, b3, w4, b4, w5, b5, w6, b6):
    import os
    from concourse.bass_utils import run_bass_kernel_spmd

    ws = [w0, w1, w2, w3, w4, w5, w6]
    bs = [b0, b1, b2, b3, b4, b5, b6]
    wpk, bpk = _pack_host(ws, bs)
    x = np.asarray(x, np.float32)
    in_maps = [{"xr": _prep_x(x[b]), "wp": wpk, "bp": bpk}
               for b in range(N_CORES)]
    nc = _build()
    trace = bool(os.environ.get("BITCONV_TRACE"))
    res = run_bass_kernel_spmd(nc, in_maps, core_ids=list(range(N_CORES)),
                               trace=trace)
    if trace:
        print(f"HW exec time: {res.exec_time_ns} ns")
        _CACHE["last_results"] = res
    return np.stack([res.results[b]["y"] for b in range(N_CORES)], axis=0)
